# revision 1
# baseline (speedup 1.0000x reference)
"""CPMAnt attention kernel for Trainium2, 8-core tensor-parallel over heads.

Math (per reference):
    q = hq @ Wq; k = hkv @ Wk; v = hkv @ Wv           (heads split col-wise)
    score = (q k^T) / sqrt(dh) + position_bias, masked
    probs = softmax(score);  out = (probs @ v) @ Wo    (Wo split row-wise)

Sharding: core c owns heads [HPC*c, HPC*(c+1)): Wq/Wk/Wv column slices,
Wo row slice, position_bias head slice. Each core returns a partial
output (its heads' contribution through Wo); the host sums the 8
partials (the "all-reduce" of the row-parallel projection).

Device layout notes:
 - hidden states are shipped pre-transposed (X^T, [D, B*S]) so the
   contraction dim D lands on SBUF partitions for the projections.
 - Q^T/K^T are produced in [dh, seq] layout (lhsT = W chunk), V in
   [seq, dh] (lhsT = X^T chunk), which feeds QK^T and PV matmuls with
   only probs needing a runtime PE transpose. V stays resident in SBUF;
   Q^T/K^T round-trip through DRAM scratch (SBUF pressure).
 - scale 1/sqrt(dh) is folded into Wq on the host; mask is folded into
   the position-bias slice as a -1e30 addend on the host (bf16).
 - bias add rides on the PE: score_psum += I^T @ pb_tile (identity
   matmul accumulating onto the QK^T bank).
 - softmax skips the max-subtraction pass (scores are O(10) here, exp
   is safe in fp32) and gets the row sum for free from the ACT
   accumulator during exp; the 1/sum scale is applied to probs rows.
 - all big matmuls run as float32r (full-rate fp32 mode at N>=256);
   every matmul operand is produced with dtype float32r end-to-end
   (walrus BIR verifier requirement).
 - W loads are split into 8 chunked DMAs to spread across queues.
"""

import contextlib
import sys

sys.path.insert(0, "/opt/trn_rl_repo")

import ml_dtypes
import numpy as np

import concourse.bacc as bacc
import concourse.mybir as mybir
import concourse.tile as tile
from concourse.masks import make_identity

F32 = mybir.dt.float32
F32R = mybir.dt.float32r
BF16 = mybir.dt.bfloat16
AF = mybir.ActivationFunctionType
P = 128
DH = 128
NEG = -1.0e30


def _r(ap):
    return ap.bitcast(F32R)


def build_program(B, S, D, HPC, nkb_tab, n_cores=8, reps=1):
    """Emit the per-core SPMD program (identical on every core).

    nkb_tab[b][qt]: number of KB-wide score blocks to compute for the
    128-row q-tile qt of batch b (blocks past the last unmasked key are
    skipped entirely; skipped probs are zero by construction).
    reps>1 wraps the body in a For_i loop (benchmarking only).
    """
    WPC = HPC * DH            # per-core projection width
    BS = B * S
    KB = min(512, S)          # score/key block width
    ST = 512                  # phase-A seq tile
    DCOL = min(512, D)        # phase-C output col block
    n_d = D // P
    n_st = BS // ST
    n_m = WPC // P
    n_sub = ST // P
    QSUP = min(512, S)        # q super-tile for transpose+PV batching
    n_qsup = S // QSUP
    n_qts = QSUP // P
    n_dcol = D // DCOL
    nkb_max = max(max(r) for r in nkb_tab)
    NCH = 8 if n_d % 8 == 0 else 1   # W-load DMA chunking

    nc = bacc.Bacc("TRN2", target_bir_lowering=False, debug=False,
                   num_devices=n_cores)
    xqT = nc.dram_tensor("xqT", [D, BS], F32R, kind="ExternalInput").ap()
    xkT = nc.dram_tensor("xkT", [D, BS], F32R, kind="ExternalInput").ap()
    wq = nc.dram_tensor("wq", [D, WPC], F32R, kind="ExternalInput").ap()
    wk = nc.dram_tensor("wk", [D, WPC], F32R, kind="ExternalInput").ap()
    wv = nc.dram_tensor("wv", [D, WPC], F32R, kind="ExternalInput").ap()
    wo = nc.dram_tensor("wo", [WPC, D], F32R, kind="ExternalInput").ap()
    pbm = nc.dram_tensor("pbm", [B, HPC, S, S], BF16,
                         kind="ExternalInput").ap()
    out = nc.dram_tensor("out", [BS, D], F32, kind="ExternalOutput").ap()

    with tile.TileContext(nc, pool_alloc_mode="queue") as tc, \
            contextlib.ExitStack() as es:
        dpool = es.enter_context(tc.tile_pool(name="dram", bufs=1,
                                              space="DRAM"))
        cpool = es.enter_context(tc.tile_pool(name="const", bufs=1))
        ctx_pool = es.enter_context(tc.tile_pool(name="ctx", bufs=1))

        qt_s = dpool.tile([WPC, BS], F32R, tag="qt_s")
        kt_s = dpool.tile([WPC, BS], F32R, tag="kt_s")
        # V stays resident in SBUF through phase B ([p, seq-tile, d])
        v_sb = ctx_pool.tile([P, BS // P, WPC], F32R, tag="v_sb")

        ident_f = cpool.tile([P, P], F32, tag="ident_f")
        make_identity(nc, ident_f[:])
        # DVE cast-copies so the verifier sees engine-rounded producers.
        ident = cpool.tile([P, P], F32R, tag="ident")
        nc.vector.tensor_copy(ident[:], ident_f[:])
        ident_bf = cpool.tile([P, P], BF16, tag="ident_bf")
        nc.vector.tensor_copy(ident_bf[:], ident_f[:])

        rep_cm = tc.For_i(0, reps, 1) if reps > 1 else contextlib.nullcontext()
        with rep_cm:
            # ---------- Phase A1: Q^T = (X Wq)^T ----------
            with (
                tc.tile_pool(name="a1w", bufs=1) as wpool,
                tc.tile_pool(name="a1x", bufs=12) as xpool,
                tc.tile_pool(name="a1s", bufs=4) as spool,
                tc.tile_pool(name="a1p", bufs=2, space="PSUM") as psa,
            ):
                wq_sb = wpool.tile([P, n_d, WPC], F32R, tag="wq_sb")
                wq_r = wq.rearrange("(a p) c -> p a c", p=P)
                for c in range(NCH):
                    sl = slice(c * n_d // NCH, (c + 1) * n_d // NCH)
                    nc.sync.dma_start(wq_sb[:, sl, :], wq_r[:, sl, :])
                for n in range(n_st):
                    ps = psa.tile([P, n_m, 512], F32, tag="ps_a")
                    for d in range(n_d):
                        xt = xpool.tile([P, ST], F32R, tag="xt")
                        nc.sync.dma_start(
                            xt[:], xqT[d * P:(d + 1) * P, n * ST:(n + 1) * ST])
                        for m in range(n_m):
                            nc.tensor.matmul(
                                ps[:, m, :ST],
                                _r(wq_sb[:, d, m * P:(m + 1) * P]),
                                _r(xt[:]),
                                start=(d == 0), stop=(d == n_d - 1))
                    for m in range(n_m):
                        st = spool.tile([P, ST], F32R, tag="st")
                        nc.scalar.copy(st[:], ps[:, m, :ST])
                        nc.sync.dma_start(
                            qt_s[m * P:(m + 1) * P, n * ST:(n + 1) * ST],
                            st[:])

            # ---------- Phase A2: K^T and V (one pass over X_kv^T) ------
            with (
                tc.tile_pool(name="a2w", bufs=1) as wpool2,
                tc.tile_pool(name="a2x", bufs=4) as xpool2,
                tc.tile_pool(name="a2s", bufs=2) as spool2,
                tc.tile_pool(name="a2pk", bufs=1, space="PSUM") as psk_pool,
                tc.tile_pool(name="a2pv", bufs=1, space="PSUM") as psv_pool,
            ):
                wk_sb = wpool2.tile([P, n_d, WPC], F32R, tag="wk_sb")
                wk_r = wk.rearrange("(a p) c -> p a c", p=P)
                for c in range(NCH):
                    sl = slice(c * n_d // NCH, (c + 1) * n_d // NCH)
                    nc.sync.dma_start(wk_sb[:, sl, :], wk_r[:, sl, :])
                wv_sb = wpool2.tile([P, n_d, WPC], F32R, tag="wv_sb")
                wv_r = wv.rearrange("(a p) c -> p a c", p=P)
                for c in range(NCH):
                    sl = slice(c * n_d // NCH, (c + 1) * n_d // NCH)
                    nc.sync.dma_start(wv_sb[:, sl, :], wv_r[:, sl, :])
                for n in range(n_st):
                    psk = psk_pool.tile([P, n_m, 512], F32, tag="ps_k")
                    psv = psv_pool.tile([P, n_sub, 512], F32, tag="ps_v")
                    for d in range(n_d):
                        xt = xpool2.tile([P, ST], F32R, tag="xt2")
                        nc.sync.dma_start(
                            xt[:], xkT[d * P:(d + 1) * P, n * ST:(n + 1) * ST])
                        for m in range(n_m):
                            nc.tensor.matmul(
                                psk[:, m, :ST],
                                _r(wk_sb[:, d, m * P:(m + 1) * P]),
                                _r(xt[:]),
                                start=(d == 0), stop=(d == n_d - 1))
                        for s2 in range(n_sub):
                            nc.tensor.matmul(
                                psv[:, s2, :WPC],
                                _r(xt[:, s2 * P:(s2 + 1) * P]),
                                _r(wv_sb[:, d, :]),
                                start=(d == 0), stop=(d == n_d - 1))
                    for m in range(n_m):
                        st = spool2.tile([P, ST], F32R, tag="stk")
                        nc.scalar.copy(st[:], psk[:, m, :ST])
                        nc.sync.dma_start(
                            kt_s[m * P:(m + 1) * P, n * ST:(n + 1) * ST],
                            st[:])
                    for s2 in range(n_sub):
                        nc.vector.tensor_copy(
                            v_sb[:, n * n_sub + s2, :WPC],
                            psv[:, s2, :WPC])
            # ---------- Phase B: attention per (b, h) ----------
            ctx_tiles = {}
            with (
                tc.tile_pool(name="bh", bufs=2) as bh_pool,
                tc.tile_pool(name="pb", bufs=7) as pb_pool,
                tc.tile_pool(name="probs", bufs=8) as probs_pool,
                tc.tile_pool(name="pt", bufs=4) as pt_pool,
                tc.tile_pool(name="rsum", bufs=8) as rsum_pool,
                tc.tile_pool(name="bps", bufs=2, space="PSUM") as psum_s,
                tc.tile_pool(name="bpt", bufs=2, space="PSUM") as psum_t,
                tc.tile_pool(name="bpc", bufs=2, space="PSUM") as psum_c,
            ):
                for b in range(B):
                    for h in range(HPC):
                        qth = bh_pool.tile([P, S], F32R, tag="qth")
                        nc.sync.dma_start(
                            qth[:],
                            qt_s[h * P:(h + 1) * P, b * S:(b + 1) * S])
                        kth = bh_pool.tile([P, S], F32R, tag="kth")
                        nc.sync.dma_start(
                            kth[:],
                            kt_s[h * P:(h + 1) * P, b * S:(b + 1) * S])
                        ctx_t = ctx_pool.tile([P, S], F32R,
                                              tag=f"ctx_{b}_{h}")
                        ctx_tiles[(b, h)] = ctx_t

                        for sup in range(n_qsup):
                            kmax_sup = max(
                                nkb_tab[b][sup * n_qts + qt]
                                for qt in range(n_qts)) * KB
                            nj = kmax_sup // P
                            probs_list = []
                            for qt in range(n_qts):
                                gqt = sup * n_qts + qt
                                nkb = nkb_tab[b][gqt]
                                q0 = gqt * P
                                pss = psum_s.tile([P, nkb_max, KB], F32,
                                                  tag="ps_s")
                                probs = probs_pool.tile([P, S], F32R,
                                                        tag="probs")
                                sums = rsum_pool.tile([P, max(2, nkb_max)],
                                                      F32, tag="sums")
                                for kb in range(nkb):
                                    nc.tensor.matmul(
                                        pss[:, kb, :],
                                        _r(qth[:, q0:q0 + P]),
                                        _r(kth[:, kb * KB:(kb + 1) * KB]),
                                        start=True, stop=False)
                                    pb_t = pb_pool.tile([P, KB], BF16,
                                                        tag="pb")
                                    nc.sync.dma_start(
                                        pb_t[:],
                                        pbm[b, h, q0:q0 + P,
                                            kb * KB:(kb + 1) * KB])
                                    nc.tensor.matmul(
                                        pss[:, kb, :], ident_bf[:],
                                        pb_t[:], start=False, stop=True)
                                    nc.scalar.activation(
                                        probs[:, kb * KB:(kb + 1) * KB],
                                        pss[:, kb, :], AF.Exp,
                                        accum_out=sums[:, kb:kb + 1])
                                if nkb * KB < kmax_sup:
                                    nc.gpsimd.memset(
                                        probs[:, nkb * KB:kmax_sup], 0.0)
                                recip = rsum_pool.tile([P, 1], F32,
                                                       tag="recip")
                                if nkb == 1:
                                    nc.vector.reciprocal(recip[:],
                                                         sums[:, 0:1])
                                else:
                                    tot = rsum_pool.tile([P, 1], F32,
                                                         tag="tot")
                                    nc.vector.tensor_add(
                                        tot[:], sums[:, 0:1], sums[:, 1:2])
                                    for kb in range(2, nkb):
                                        nc.vector.tensor_add(
                                            tot[:], tot[:],
                                            sums[:, kb:kb + 1])
                                    nc.vector.reciprocal(recip[:], tot[:])
                                nc.vector.tensor_scalar_mul(
                                    probs[:, :nkb * KB],
                                    probs[:, :nkb * KB], recip[:])
                                probs_list.append(probs)

                            ps_ctx = psum_c.tile([P, QSUP], F32,
                                                 tag="ps_ctx")
                            for j in range(nj):
                                # first q-tile whose computed k-range covers
                                # block j; earlier q-tiles have zero probs
                                # there and can skip transpose+PV entirely.
                                vq = 0
                                while (vq < n_qts and
                                       nkb_tab[b][sup * n_qts + vq] * KB
                                       <= j * P):
                                    vq += 1
                                if vq >= n_qts:
                                    continue
                                # keep PV free-dim >=256 (f32r full rate)
                                if (n_qts - vq) * P < 256:
                                    vq = max(0, n_qts - 256 // P)
                                ps_t = psum_t.tile([P, QSUP], F32,
                                                   tag="ps_t")
                                for qt in range(vq, n_qts):
                                    nc.tensor.transpose(
                                        _r(ps_t[:, qt * P:(qt + 1) * P]),
                                        probs_list[qt][:, j * P:(j + 1) * P],
                                        ident[:])
                                pT = pt_pool.tile([P, QSUP], F32R, tag="pT")
                                nc.vector.tensor_copy(pT[:, vq * P:],
                                                      ps_t[:, vq * P:])
                                nc.tensor.matmul(
                                    ps_ctx[:, vq * P:],
                                    _r(v_sb[:, b * (S // P) + j,
                                            h * DH:(h + 1) * DH]),
                                    _r(pT[:, vq * P:]),
                                    start=(j == 0), stop=(j == nj - 1))
                            nc.vector.tensor_copy(
                                ctx_t[:, sup * QSUP:(sup + 1) * QSUP],
                                ps_ctx[:])

            # ---------- Phase C: out = ctx @ Wo (partial over heads) -----
            with (
                tc.tile_pool(name="co", bufs=4) as opool,
                tc.tile_pool(name="cw", bufs=1) as wopool,
                tc.tile_pool(name="cp", bufs=4, space="PSUM") as psum_o,
            ):
                wo_sb = wopool.tile([P, HPC, D], F32R, tag="wo_sb")
                wo_r = wo.rearrange("(h p) d -> p h d", p=P)
                for c in range(8):
                    sl = slice(c * D // 8, (c + 1) * D // 8)
                    nc.sync.dma_start(wo_sb[:, :, sl], wo_r[:, :, sl])
                for b in range(B):
                    for gqt in range(S // P):
                        for dc in range(n_dcol):
                            pso = psum_o.tile([P, DCOL], F32, tag="ps_o")
                            for h in range(HPC):
                                nc.tensor.matmul(
                                    pso[:],
                                    _r(ctx_tiles[(b, h)]
                                       [:, gqt * P:(gqt + 1) * P]),
                                    _r(wo_sb[:, h, dc * DCOL:(dc + 1) * DCOL]),
                                    start=(h == 0), stop=(h == HPC - 1))
                            ost = opool.tile([P, DCOL], F32, tag="ost")
                            if (gqt + dc) % 2 == 0:
                                nc.scalar.copy(ost[:], pso[:])
                            else:
                                nc.vector.tensor_copy(ost[:], pso[:])
                            nc.sync.dma_start(
                                out[b * S + gqt * P:b * S + (gqt + 1) * P,
                                    dc * DCOL:(dc + 1) * DCOL], ost[:])

    nc.compile()
    return nc


def causal_nkb_tab(mask, KB):
    """nkb_tab from the actual bool mask [B, S, S] (general, not just tril)."""
    B, S, _ = mask.shape
    tab = []
    for b in range(B):
        row = []
        for qt in range(S // P):
            m = mask[b, qt * P:(qt + 1) * P, :]
            anyk = np.nonzero(m.any(axis=0))[0]
            last = int(anyk[-1]) if len(anyk) else 0
            row.append(last // KB + 1)
        tab.append(row)
    return tab


def shard_inputs(hidden_q, hidden_kv, attention_mask, position_bias,
                 Wq, Wk, Wv, Wo, n_cores=8):
    hidden_q = np.asarray(hidden_q, np.float32)
    hidden_kv = np.asarray(hidden_kv, np.float32)
    attention_mask = np.asarray(attention_mask, bool)
    position_bias = np.asarray(position_bias, np.float32)
    Wq = np.asarray(Wq, np.float32)
    Wk = np.asarray(Wk, np.float32)
    Wv = np.asarray(Wv, np.float32)
    Wo = np.asarray(Wo, np.float32)

    B, S, D = hidden_q.shape
    H = position_bias.shape[1]
    HPC = H // n_cores
    WPC = HPC * DH
    scale = np.float32(1.0 / np.sqrt(DH))

    xq = np.ascontiguousarray(hidden_q.reshape(B * S, D).T)
    xk = np.ascontiguousarray(hidden_kv.reshape(B * S, D).T)
    mask_add = np.where(attention_mask, np.float32(0.0),
                        np.float32(NEG))[:, None]   # [B,1,S,S]
    Wq_s = Wq * scale

    in_maps = []
    for c in range(n_cores):
        sl = slice(c * WPC, (c + 1) * WPC)
        pbm = (position_bias[:, c * HPC:(c + 1) * HPC] + mask_add)
        pbm = pbm.astype(ml_dtypes.bfloat16)
        in_maps.append({
            "xqT": xq,
            "xkT": xk,
            "wq": np.ascontiguousarray(Wq_s[:, sl]),
            "wk": np.ascontiguousarray(Wk[:, sl]),
            "wv": np.ascontiguousarray(Wv[:, sl]),
            "wo": np.ascontiguousarray(Wo[sl, :]),
            "pbm": np.ascontiguousarray(pbm),
        })
    meta = dict(B=B, S=S, D=D, HPC=HPC,
                nkb_tab=causal_nkb_tab(attention_mask, min(512, S)))
    return in_maps, meta


_PROG_CACHE = {}


def _get_program(B, S, D, HPC, nkb_key, n_cores):
    key = (B, S, D, HPC, nkb_key, n_cores)
    if key not in _PROG_CACHE:
        _PROG_CACHE[key] = build_program(
            B, S, D, HPC, [list(r) for r in nkb_key], n_cores)
    return _PROG_CACHE[key]


def kernel(hidden_q, hidden_kv, attention_mask, position_bias,
           Wq, Wk, Wv, Wo):
    n_cores = 8
    in_maps, meta = shard_inputs(hidden_q, hidden_kv, attention_mask,
                                 position_bias, Wq, Wk, Wv, Wo, n_cores)
    nkb_key = tuple(tuple(r) for r in meta["nkb_tab"])
    nc = _get_program(meta["B"], meta["S"], meta["D"], meta["HPC"],
                      nkb_key, n_cores)

    from concourse.bass_utils import run_bass_kernel_spmd
    res = None
    for attempt in range(3):
        try:
            res = run_bass_kernel_spmd(nc, in_maps, list(range(n_cores)))
            break
        except Exception:
            # Transient NRT_EXEC_UNIT_UNRECOVERABLE wedges recover on a
            # fresh PJRT client; reset backends and retry.
            if attempt == 2:
                raise
            try:
                import time as _time

                import jax as _jax
                _jax.clear_caches()
                _jax.extend.backend.clear_backends()
                _time.sleep(15 * (attempt + 1))
            except Exception:
                pass

    B, S, D = meta["B"], meta["S"], meta["D"]
    acc = np.zeros((B * S, D), np.float32)
    for r in res.results:
        acc += r["out"]
    return acc.reshape(B, S, D)

